# revision 9
# baseline (speedup 1.0000x reference)
"""Trainium2 Bass kernel for CustomRNN: h_t = tanh(x_t @ W + b + h_{t-1} @ U).

Strategy: data-parallel over batch across 8 NeuronCores (8 batch rows per
core, W/U/b replicated). Inside each core the recurrence runs fully
transposed (h kept as h^T [H-on-partitions x batch]) so no per-step
transposes are needed; U is the stationary matmul operand in fp16 (fast
weight load), PSUM accumulates fp32. The input projection x@W+b is
precomputed per time-chunk with big matmuls into an fp16 staging buffer and
injected into the recurrence PSUM via an identity-weights matmul, so the
only on-chain work per step is TensorE matmuls plus one tanh per
half-state on ScalarE (two halves pipelined against the matmul stream).
The fp32 output is produced off the critical chain by bulk VectorE casts of
the fp16 hidden states, DMA'd out per chunk.

Host side pre-transposes x to [I, S, B_core] per shard (and re-transposes
the [H, S, B_core] output) so every DMA is a full-bandwidth partition-major
transfer.
"""

import itertools
import os
import sys

import numpy as np

sys.path.insert(0, "/opt/trn_rl_repo")

B, S_FULL, I, H = 64, 2048, 512, 512
N_CORES = 8
B_CORE = B // N_CORES  # 8

S = int(os.environ.get("RNN_S", S_FULL))
T = int(os.environ.get("RNN_T", 128))  # timesteps per chunk
NCHUNK = S // T
assert S % T == 0 and T % 32 == 0

_CACHE = {}


def _build_program(repeat=1):
    import concourse.bass as bass
    import concourse.tile as tile
    from concourse import bacc, mybir
    from concourse.bass import ds, ts
    from concourse.masks import make_identity

    f32 = mybir.dt.float32
    f16 = mybir.dt.float16
    AF = mybir.ActivationFunctionType
    TB = T * B_CORE  # free-dim columns per chunk (t, b) pairs
    NSPAN = TB // 512  # 512-col spans per chunk for phase-1 matmuls

    nc = bacc.Bacc(
        "TRN2", target_bir_lowering=False, debug=False, num_devices=N_CORES
    )
    xT_d = nc.dram_tensor("xT", [I, S, B_CORE], f32, kind="ExternalInput")
    w_d = nc.dram_tensor("w", [I, H], f32, kind="ExternalInput")
    u_d = nc.dram_tensor("u", [H, H], f32, kind="ExternalInput")
    b_d = nc.dram_tensor("b", [H], f32, kind="ExternalInput")
    oT_d = nc.dram_tensor("outT", [H, S, B_CORE], f32, kind="ExternalOutput")

    with tile.TileContext(nc) as tc:
        with (
            tc.tile_pool(name="const", bufs=1) as const_pool,
            tc.tile_pool(name="xt32", bufs=2) as xt32_pool,
            tc.tile_pool(name="xt16", bufs=2) as xt16_pool,
            tc.tile_pool(name="xw", bufs=2) as xw_pool,
            tc.tile_pool(name="outst", bufs=2) as out_pool,
            tc.tile_pool(name="hst", bufs=2) as h_pool,
            tc.tile_pool(name="p1ps", bufs=2, space=bass.MemorySpace.PSUM) as p1_psum,
            tc.tile_pool(name="rps", bufs=2, space=bass.MemorySpace.PSUM) as r_psum,
        ):
            # ---- constants: W, U (fp16), bias, identity ----
            w16 = const_pool.tile([128, 4, 512], f16, tag="w16")
            u16 = const_pool.tile([128, 4, 512], f16, tag="u16")
            bias_sb = const_pool.tile([128, 4], f32, tag="bias")
            ident16 = const_pool.tile([128, 128], f16, tag="ident")
            nc.sync.dma_start(bias_sb[:, :], b_d.ap().rearrange("(j p) -> p j", p=128))
            with tc.tile_pool(name="stage", bufs=2) as stage_pool:
                id32 = stage_pool.tile([128, 128], f32, tag="id32")
                make_identity(nc, id32[:, :])
                nc.vector.tensor_copy(ident16[:, :], id32[:, :])
                w32 = stage_pool.tile([128, 4, 512], f32, tag="wu32")
                for ic in range(4):
                    nc.sync.dma_start(w32[:, ic, :], w_d[ts(ic, 128), :])
                nc.vector.tensor_copy(
                    w16[:, :, :].rearrange("p a c -> p (a c)"),
                    w32[:, :, :].rearrange("p a c -> p (a c)"),
                )
                u32 = stage_pool.tile([128, 4, 512], f32, tag="wu32")
                for ic in range(4):
                    nc.sync.dma_start(u32[:, ic, :], u_d[ts(ic, 128), :])
                nc.vector.tensor_copy(
                    u16[:, :, :].rearrange("p a c -> p (a c)"),
                    u32[:, :, :].rearrange("p a c -> p (a c)"),
                )

            hst_prev = None
            for rep, c in itertools.product(range(repeat), range(NCHUNK)):
                if c == 0:
                    hst_prev = None
                t0 = c * T
                # ---- load + cast x^T chunk ----
                xt32 = xt32_pool.tile([128, 4, TB], f32, tag="xt32")
                xt16 = xt16_pool.tile([128, 4, TB], f16, tag="xt16")
                for ic in range(4):
                    nc.sync.dma_start(
                        xt32[:, ic, :],
                        xT_d[ts(ic, 128), t0 : t0 + T, :].rearrange(
                            "p t b -> p (t b)"
                        ),
                    )
                    nc.vector.tensor_copy(xt16[:, ic, :], xt32[:, ic, :])

                # ---- phase 1: xw^T = W^T x^T (+bias via ACT copy), fp16 out ----
                xw16 = xw_pool.tile([128, 4, TB], f16, tag="xw16")
                for j in range(4):
                    for n in range(NSPAN):
                        ps1 = p1_psum.tile([128, 512], f32, tag="p1")
                        for i in range(4):
                            nc.tensor.matmul(
                                ps1[:, :],
                                w16[:, i, ts(j, 128)],
                                xt16[:, i, ts(n, 512)],
                                start=(i == 0),
                                stop=(i == 3),
                            )
                        nc.scalar.activation(
                            xw16[:, j, ts(n, 512)],
                            ps1[:, :],
                            AF.Identity,
                            bias=bias_sb[:, j : j + 1],
                        )

                # ---- recurrence over T steps ----
                outst = out_pool.tile([128, 4, TB], f32, tag="outst")
                hst = h_pool.tile([128, T, 4, B_CORE], f16, tag="hst")

                for t in range(T):
                    gt = t0 + t

                    def h_rhs(k):
                        if t > 0:
                            return hst[:, t - 1, k, :]
                        return hst_prev[:, T - 1, k, :]

                    if gt == 0:
                        nc.scalar.activation(
                            hst[:, t, 0:2, :], xw16[:, 0:2, ts(t, 8)], AF.Tanh
                        )
                        nc.scalar.activation(
                            hst[:, t, 2:4, :], xw16[:, 2:4, ts(t, 8)], AF.Tanh
                        )
                    else:
                        for jlo, tagn in ((0, "psA"), (2, "psB")):
                            psb = r_psum.tile(
                                [128, 2, B_CORE], f32, tag=tagn, name=tagn
                            )
                            nc.tensor.matmul(
                                psb[:, :, :],
                                ident16[:, :],
                                xw16[:, jlo : jlo + 2, ts(t, 8)],
                                start=True,
                                stop=False,
                                skip_group_check=True,
                            )
                            for j in (jlo, jlo + 1):
                                for k in range(4):
                                    nc.tensor.matmul(
                                        psb[:, j - jlo, :],
                                        u16[:, k, ts(j, 128)],
                                        h_rhs(k),
                                        start=False,
                                        stop=(j == jlo + 1 and k == 3),
                                        skip_group_check=True,
                                    )
                            nc.scalar.activation(
                                hst[:, t, jlo : jlo + 2, :], psb[:, :, :], AF.Tanh
                            )

                    # bulk fp32 output casts per 32-step span (off-chain, DVE)
                    if t % 32 == 31:
                        m = t // 32
                        for j in range(4):
                            nc.vector.tensor_copy(
                                outst[:, j, ds(m * 256, 256)],
                                hst[:, ds(m * 32, 32), j, :].rearrange(
                                    "p t b -> p (t b)"
                                ),
                            )
                hst_prev = hst

                # ---- store chunk ----
                for j in range(4):
                    nc.sync.dma_start(
                        oT_d[ts(j, 128), t0 : t0 + T, :].rearrange("p t b -> p (t b)"),
                        outst[:, j, :],
                    )

    nc.compile()
    return nc


def _get_program(repeat=1):
    key = f"nc{repeat}"
    if key not in _CACHE:
        _CACHE[key] = _build_program(repeat)
    return _CACHE[key]


def kernel(x, W_i, U_i, b_i, trace=False, repeat=1):
    from concourse.bass_utils import run_bass_kernel_spmd

    nc = _get_program(repeat)
    x = np.asarray(x, dtype=np.float32)
    W_i = np.asarray(W_i, dtype=np.float32)
    U_i = np.asarray(U_i, dtype=np.float32)
    b_i = np.asarray(b_i, dtype=np.float32)

    in_maps = []
    for ci in range(N_CORES):
        shard = x[ci * B_CORE : (ci + 1) * B_CORE, :S]  # [B_CORE, S, I]
        xT = np.ascontiguousarray(shard.transpose(2, 1, 0))  # [I, S, B_CORE]
        in_maps.append({"xT": xT, "w": W_i, "u": U_i, "b": b_i})

    res = run_bass_kernel_spmd(
        nc, in_maps, list(range(N_CORES)), trace=trace
    )
    _CACHE["last_result"] = res

    hidden = np.empty((B, S, H), dtype=np.float32)
    for ci in range(N_CORES):
        oT = res.results[ci]["outT"]  # [H, S, B_CORE]
        hidden[ci * B_CORE : (ci + 1) * B_CORE] = oT.transpose(2, 1, 0)
    h_last = np.ascontiguousarray(hidden[:, -1, :])
    return hidden, h_last


# revision 10
# speedup vs baseline: 1.2788x; 1.2788x over previous
"""Trainium2 Bass kernel for CustomRNN: h_t = tanh(x_t @ W + b + h_{t-1} @ U).

Strategy: data-parallel over batch across 8 NeuronCores (8 batch rows per
core, W/U/b replicated). Inside each core the recurrence runs fully
transposed (h kept as h^T [H-on-partitions x batch]) so no per-step
transposes are needed; U is the stationary matmul operand in fp16 (fast
weight load), PSUM accumulates fp32. The input projection x@W+b is
precomputed per time-chunk with big matmuls into an fp16 staging buffer and
injected into the recurrence PSUM via an identity-weights matmul, so the
only on-chain work per step is TensorE matmuls plus one tanh per
half-state on ScalarE (two halves pipelined against the matmul stream).
The fp32 output is produced off the critical chain by bulk VectorE casts of
the fp16 hidden states, DMA'd out per chunk.

Host side pre-transposes x to [I, S, B_core] per shard (and re-transposes
the [H, S, B_core] output) so every DMA is a full-bandwidth partition-major
transfer.
"""

import itertools
import os
import sys

import numpy as np

sys.path.insert(0, "/opt/trn_rl_repo")

B, S_FULL, I, H = 64, 2048, 512, 512
N_CORES = 8
B_CORE = B // N_CORES  # 8

S = int(os.environ.get("RNN_S", S_FULL))
T = int(os.environ.get("RNN_T", 128))  # timesteps per chunk
NCHUNK = S // T
assert S % T == 0 and T % 32 == 0

_CACHE = {}


def _build_program(repeat=1):
    import concourse.bass as bass
    import concourse.tile as tile
    from concourse import bacc, mybir
    from concourse.bass import ds, ts
    from concourse.masks import make_identity

    f32 = mybir.dt.float32
    f16 = mybir.dt.float16
    AF = mybir.ActivationFunctionType
    TB = T * B_CORE  # free-dim columns per chunk (t, b) pairs
    NSPAN = TB // 512  # 512-col spans per chunk for phase-1 matmuls

    nc = bacc.Bacc(
        "TRN2", target_bir_lowering=False, debug=False, num_devices=N_CORES
    )
    xT_d = nc.dram_tensor("xT", [I, S, B_CORE], f32, kind="ExternalInput")
    w_d = nc.dram_tensor("w", [I, H], f32, kind="ExternalInput")
    u_d = nc.dram_tensor("u", [H, H], f32, kind="ExternalInput")
    b_d = nc.dram_tensor("b", [H], f32, kind="ExternalInput")
    oT_d = nc.dram_tensor("outT", [H, S, B_CORE], f32, kind="ExternalOutput")

    with tile.TileContext(nc) as tc:
        with (
            tc.tile_pool(name="const", bufs=1) as const_pool,
            tc.tile_pool(name="xt32", bufs=2) as xt32_pool,
            tc.tile_pool(name="xt16", bufs=2) as xt16_pool,
            tc.tile_pool(name="xw", bufs=2) as xw_pool,
            tc.tile_pool(name="outst", bufs=2) as out_pool,
            tc.tile_pool(name="hst", bufs=2) as h_pool,
            tc.tile_pool(name="p1ps", bufs=2, space=bass.MemorySpace.PSUM) as p1_psum,
            tc.tile_pool(name="rps", bufs=2, space=bass.MemorySpace.PSUM) as r_psum,
        ):
            # ---- constants: W, U (fp16), bias, identity ----
            w16 = const_pool.tile([128, 4, 512], f16, tag="w16")
            u16 = const_pool.tile([128, 4, 512], f16, tag="u16")
            bias_sb = const_pool.tile([128, 4], f32, tag="bias")
            ident16 = const_pool.tile([128, 128], f16, tag="ident")
            nc.sync.dma_start(bias_sb[:, :], b_d.ap().rearrange("(j p) -> p j", p=128))
            with tc.tile_pool(name="stage", bufs=2) as stage_pool:
                id32 = stage_pool.tile([128, 128], f32, tag="id32")
                make_identity(nc, id32[:, :])
                nc.vector.tensor_copy(ident16[:, :], id32[:, :])
                w32 = stage_pool.tile([128, 4, 512], f32, tag="wu32")
                for ic in range(4):
                    nc.sync.dma_start(w32[:, ic, :], w_d[ts(ic, 128), :])
                nc.vector.tensor_copy(
                    w16[:, :, :].rearrange("p a c -> p (a c)"),
                    w32[:, :, :].rearrange("p a c -> p (a c)"),
                )
                u32 = stage_pool.tile([128, 4, 512], f32, tag="wu32")
                for ic in range(4):
                    nc.sync.dma_start(u32[:, ic, :], u_d[ts(ic, 128), :])
                nc.vector.tensor_copy(
                    u16[:, :, :].rearrange("p a c -> p (a c)"),
                    u32[:, :, :].rearrange("p a c -> p (a c)"),
                )

            hst_prev = None
            for rep, c in itertools.product(range(repeat), range(NCHUNK)):
                if c == 0:
                    hst_prev = None
                t0 = c * T
                # ---- load + cast x^T chunk ----
                xt32 = xt32_pool.tile([128, 4, TB], f32, tag="xt32")
                xt16 = xt16_pool.tile([128, 4, TB], f16, tag="xt16")
                for ic in range(4):
                    nc.sync.dma_start(
                        xt32[:, ic, :],
                        xT_d[ts(ic, 128), t0 : t0 + T, :].rearrange(
                            "p t b -> p (t b)"
                        ),
                    )
                    nc.vector.tensor_copy(xt16[:, ic, :], xt32[:, ic, :])

                # ---- phase 1: xw^T = W^T x^T (+bias via ACT copy), fp16 out ----
                xw16 = xw_pool.tile([128, 4, TB], f16, tag="xw16")
                for j in range(4):
                    for n in range(NSPAN):
                        ps1 = p1_psum.tile([128, 512], f32, tag="p1")
                        for i in range(4):
                            nc.tensor.matmul(
                                ps1[:, :],
                                w16[:, i, ts(j, 128)],
                                xt16[:, i, ts(n, 512)],
                                start=(i == 0),
                                stop=(i == 3),
                            )
                        nc.scalar.activation(
                            xw16[:, j, ts(n, 512)],
                            ps1[:, :],
                            AF.Identity,
                            bias=bias_sb[:, j : j + 1],
                        )

                # ---- recurrence over T steps ----
                outst = out_pool.tile([128, 4, TB], f32, tag="outst")
                hst = h_pool.tile([128, T, 4, B_CORE], f16, tag="hst")

                for t in range(T):
                    gt = t0 + t

                    def h_rhs(k):
                        if t > 0:
                            return hst[:, t - 1, k, :]
                        return hst_prev[:, T - 1, k, :]

                    if gt == 0:
                        nc.scalar.activation(
                            hst[:, t, 0:2, :], xw16[:, 0:2, ts(t, 8)], AF.Tanh
                        )
                        nc.scalar.activation(
                            hst[:, t, 2:4, :], xw16[:, 2:4, ts(t, 8)], AF.Tanh
                        )
                    else:
                        for jlo, tagn in ((0, "psA"), (2, "psB")):
                            psb = r_psum.tile(
                                [128, 2, B_CORE], f32, tag=tagn, name=tagn
                            )
                            nc.tensor.matmul(
                                psb[:, :, :],
                                ident16[:, :],
                                xw16[:, jlo : jlo + 2, ts(t, 8)],
                                start=True,
                                stop=False,
                                skip_group_check=True,
                            )
                            for j in (jlo, jlo + 1):
                                for k in range(4):
                                    nc.tensor.matmul(
                                        psb[:, j - jlo, :],
                                        u16[:, k, ts(j, 128)],
                                        h_rhs(k),
                                        start=False,
                                        stop=(j == jlo + 1 and k == 3),
                                        skip_group_check=True,
                                    )
                            nc.scalar.activation(
                                hst[:, t, jlo : jlo + 2, :], psb[:, :, :], AF.Tanh
                            )

                    # bulk fp32 output casts per 32-step span (off-chain, DVE)
                    if t % 32 == 31:
                        m = t // 32
                        for j in range(4):
                            nc.vector.tensor_copy(
                                outst[:, j, ds(m * 256, 256)].rearrange(
                                    "p (t b) -> p t b", b=B_CORE
                                ),
                                hst[:, ds(m * 32, 32), j, :],
                            )
                hst_prev = hst

                # ---- store chunk ----
                for j in range(4):
                    nc.sync.dma_start(
                        oT_d[ts(j, 128), t0 : t0 + T, :].rearrange("p t b -> p (t b)"),
                        outst[:, j, :],
                    )

    nc.compile()
    return nc


def _get_program(repeat=1):
    key = f"nc{repeat}"
    if key not in _CACHE:
        _CACHE[key] = _build_program(repeat)
    return _CACHE[key]


def kernel(x, W_i, U_i, b_i, trace=False, repeat=1):
    from concourse.bass_utils import run_bass_kernel_spmd

    nc = _get_program(repeat)
    x = np.asarray(x, dtype=np.float32)
    W_i = np.asarray(W_i, dtype=np.float32)
    U_i = np.asarray(U_i, dtype=np.float32)
    b_i = np.asarray(b_i, dtype=np.float32)

    in_maps = []
    for ci in range(N_CORES):
        shard = x[ci * B_CORE : (ci + 1) * B_CORE, :S]  # [B_CORE, S, I]
        xT = np.ascontiguousarray(shard.transpose(2, 1, 0))  # [I, S, B_CORE]
        in_maps.append({"xT": xT, "w": W_i, "u": U_i, "b": b_i})

    res = run_bass_kernel_spmd(
        nc, in_maps, list(range(N_CORES)), trace=trace
    )
    _CACHE["last_result"] = res

    hidden = np.empty((B, S, H), dtype=np.float32)
    for ci in range(N_CORES):
        oT = res.results[ci]["outT"]  # [H, S, B_CORE]
        hidden[ci * B_CORE : (ci + 1) * B_CORE] = oT.transpose(2, 1, 0)
    h_last = np.ascontiguousarray(hidden[:, -1, :])
    return hidden, h_last
